# revision 11
# baseline (speedup 1.0000x reference)
"""Trainium2 Bass kernel for nn_Conv2d_NN (retrieval_knn).

Computation: for each of T=64*64 tokens, gather its K=9 nearest spatial
neighbors (by a fixed coordinate-similarity top-k whose indices are
input-independent) and mix them with a Conv1d(kernel=K, stride=K).

Strategy:
  - The [T,T] similarity/topk depends only on the constant coordinate grid,
    so idx[T,9] is computed once on the host (replicating the reference's
    exact jax op sequence on the default backend so f32 tie-breaking
    matches the reference bit-for-bit).
  - Device kernel (SPMD over 8 cores, T sequence-sharded into 512-token
    slabs): load x window [128=(b,ci), 772] into SBUF, ap_gather the 8
    non-self neighbor slots (indices shared across partitions), then 9
    PSUM-accumulated matmuls per batch-pair with block-diag weights,
    bias via ScalarE activation, DMA out.
"""

import numpy as np

# problem constants (hardcoded per harness contract)
B, C_IN, C_OUT, HH, WW, K = 4, 32, 64, 64, 64, 9
T = HH * WW          # 4096
SIGMA = 0.1
NCORES = 8
SLAB = T // NCORES   # 512
PAIRS = 2            # batch pairs per core (2 batches each -> 128 psum rows)

_CACHE = {}


def _get_idx() -> np.ndarray:
    """Replicate the reference's coords->sim->top_k exactly, as eager jax ops
    on the CPU backend (the reference's gather cannot compile on the neuron
    backend, so the oracle necessarily runs on jax-CPU; running the same op
    sequence there makes the f32 tie-breaking in top_k match bit-for-bit)."""
    if "idx" in _CACHE:
        return _CACHE["idx"]
    import jax
    import jax.numpy as jnp

    with jax.default_device(jax.devices("cpu")[0]):
        y = jnp.linspace(-1.0, 1.0, HH)
        x = jnp.linspace(-1.0, 1.0, WW)
        yy, xx = jnp.meshgrid(y, x, indexing="ij")
        coords = jnp.stack((xx, yy), axis=0).reshape(2, T)
        sq = jnp.sum(coords * coords, axis=0)
        d2 = sq[:, None] + sq[None, :] - 2.0 * (coords.T @ coords)
        dist = jnp.sqrt(jnp.maximum(d2, 0.0) + 1e-8)
        sim = jnp.exp(-(dist * dist) / (2.0 * SIGMA * SIGMA))
        _, idx = jax.lax.top_k(sim, K)
        idx = np.asarray(idx).astype(np.int32)
    _CACHE["idx"] = idx
    return idx


def _build_program_dg(mm_dtype: str, loop_n: int = 0):
    """dma_gather-based variant: gather 512B rows of host-pre-transposed
    xt[T,128] with the SDMA engines, PE-transpose 128x128 blocks back to
    [(b,ci), (t,k)] layout, then the usual 18 PSUM-accumulated matmuls."""
    import concourse.bacc as bacc
    import concourse.tile as tile
    from concourse import mybir

    f32 = mybir.dt.float32
    mmdt = {"f32": mybir.dt.float32, "f32r": mybir.dt.float32r}[mm_dtype]
    NG = (K - 1) * SLAB          # 4096 gathered rows per slab
    QB = NG // 128               # 32 transpose blocks

    nc = bacc.Bacc("TRN2", target_bir_lowering=False, debug=False)
    xt_d = nc.dram_tensor("xt", [T, 128], f32, kind="ExternalInput").ap()
    xc_d = nc.dram_tensor("xwin", [128, SLAB], f32, kind="ExternalInput").ap()
    w_d = nc.dram_tensor("wts", [128, K * 128], f32, kind="ExternalInput").ap()
    b_d = nc.dram_tensor("bias", [128, 1], f32, kind="ExternalInput").ap()
    i_d = nc.dram_tensor("idxs", [128, NG // 16], mybir.dt.int16,
                         kind="ExternalInput").ap()
    id_d = nc.dram_tensor("ident", [128, 128], f32, kind="ExternalInput").ap()
    o_d = nc.dram_tensor("out", [PAIRS, 128, SLAB], f32,
                         kind="ExternalOutput").ap()

    with tile.TileContext(nc) as tc:
        with (
            tc.tile_pool(name="sb", bufs=1) as pool,
            tc.tile_pool(name="ps", bufs=1, space="PSUM") as ppool,
            tc.tile_pool(name="pt", bufs=4, space="PSUM") as tpool,
        ):
            Wt = pool.tile([128, K * 128], f32, tag="Wt")
            nc.sync.dma_start(Wt[:], w_d[:])
            bias = pool.tile([128, 1], f32, tag="bias")
            nc.sync.dma_start(bias[:], b_d[:])
            idxs = pool.tile([128, NG // 16], mybir.dt.int16, tag="idxs")
            nc.sync.dma_start(idxs[:], i_d[:])
            ident = pool.tile([128, 128], f32, tag="ident")
            nc.sync.dma_start(ident[:], id_d[:])

            def body():
                X = pool.tile([128, SLAB], f32, tag="X")
                nc.sync.dma_start(X[:], xc_d[:])
                G = pool.tile([128, NG], f32, tag="G")
                G3 = G[:].rearrange("p (q e) -> p q e", e=128)
                nc.gpsimd.dma_gather(G3, xt_d[:], idxs[:], num_idxs=NG,
                                     num_idxs_reg=NG, elem_size=128)
                Y = pool.tile([128, NG], f32, tag="Yd")
                for q in range(QB):
                    pt = tpool.tile([128, 128], f32, tag="pt")
                    nc.tensor.transpose(pt[:], G3[:, q, :], ident[:])
                    if q % 2 == 0:
                        nc.vector.tensor_copy(Y[:, q * 128:(q + 1) * 128], pt[:])
                    else:
                        nc.scalar.copy(Y[:, q * 128:(q + 1) * 128], pt[:])

                for p in range(PAIRS):
                    ps = ppool.tile([128, SLAB], f32, tag=f"ps{p}")
                    for k in range(K):
                        if k == 0:
                            rhs = X[64 * p:64 * p + 64, :]
                        else:
                            rhs = Y[64 * p:64 * p + 64,
                                    (k - 1) * SLAB:k * SLAB]
                        lhsT = Wt[64 * p:64 * p + 64, k * 128:(k + 1) * 128]
                        if mm_dtype == "f32r":
                            rhs = rhs.bitcast(mmdt)
                            lhsT = lhsT.bitcast(mmdt)
                        nc.tensor.matmul(ps[:], lhsT=lhsT, rhs=rhs,
                                         start=(k == 0), stop=(k == K - 1))
                    ob = pool.tile([128, SLAB], f32, tag=f"ob{p}")
                    nc.scalar.activation(ob[:], ps[:],
                                         mybir.ActivationFunctionType.Identity,
                                         bias=bias[:])
                    nc.sync.dma_start(o_d[p], ob[:])

            if loop_n:
                with tc.For_i(0, loop_n, 1):
                    body()
            else:
                body()

    nc.compile()
    return nc


def _make_in_maps_dg(x, conv_w, conv_b, idx):
    xflat = np.ascontiguousarray(x.reshape(B * C_IN, T), dtype=np.float32)
    xt = np.ascontiguousarray(xflat.T)                       # [T, 128]

    wT = np.ascontiguousarray(conv_w.transpose(1, 0, 2), dtype=np.float32)
    wts = np.zeros((64, K, 128), dtype=np.float32)
    for k in range(K):
        wts[0:32, k, 0:64] = wT[:, :, k]
        wts[32:64, k, 64:128] = wT[:, :, k]
    wts = np.concatenate([wts, wts], axis=0).reshape(128, K * 128)
    bias = np.concatenate([conv_b, conv_b]).astype(np.float32)[:, None]
    ident = np.eye(128, dtype=np.float32)

    NG = (K - 1) * SLAB
    in_maps = []
    for g in range(NCORES):
        t0 = g * SLAB
        flat = idx[t0:t0 + SLAB, 1:K].T.reshape(NG).astype(np.int16)  # k-major
        wrapped = flat.reshape(NG // 16, 16).T                        # [16, NG/16]
        iw = np.ascontiguousarray(np.tile(wrapped, (8, 1)), dtype=np.int16)
        in_maps.append({
            "xt": xt,
            "xwin": np.ascontiguousarray(xflat[:, t0:t0 + SLAB]),
            "wts": wts, "bias": bias, "idxs": iw, "ident": ident,
        })
    return in_maps


def _build_program(halo: int, gather0: bool, mm_dtype: str,
                   reps: int = 1, no_gather: bool = False, loop_n: int = 0,
                   merge_gather: bool = False):
    """Build + compile the SPMD Bass program. Returns (nc, wwin).

    reps > 1 repeats the whole compute body (x-DMA, gathers, matmuls, out-DMA)
    for wall-clock slope timing; no_gather replaces gathered rhs with shifted
    X reads (wrong results, used only to time the gather contribution)."""
    import concourse.bacc as bacc
    import concourse.tile as tile
    from concourse import mybir

    wwin = SLAB + 2 * halo
    f32 = mybir.dt.float32
    mmdt = {"f32": mybir.dt.float32, "f32r": mybir.dt.float32r,
            "bf16": mybir.dt.bfloat16}[mm_dtype]

    nc = bacc.Bacc("TRN2", target_bir_lowering=False, debug=False)
    xwin_d = nc.dram_tensor("xwin", [128, wwin], f32, kind="ExternalInput").ap()
    w_d = nc.dram_tensor("wts", [128, K * 128], f32, kind="ExternalInput").ap()
    b_d = nc.dram_tensor("bias", [128, 1], f32, kind="ExternalInput").ap()
    i_d = nc.dram_tensor("idxs", [128, K * (SLAB // 16)], mybir.dt.int16,
                         kind="ExternalInput").ap()
    o_d = nc.dram_tensor("out", [PAIRS, 128, SLAB], f32,
                         kind="ExternalOutput").ap()

    with tile.TileContext(nc) as tc:
        with (
            tc.tile_pool(name="sb", bufs=1) as pool,
            tc.tile_pool(name="ps", bufs=1, space="PSUM") as ppool,
        ):
            Wt = pool.tile([128, K * 128], f32, tag="Wt")
            nc.sync.dma_start(Wt[:], w_d[:])
            bias = pool.tile([128, 1], f32, tag="bias")
            nc.sync.dma_start(bias[:], b_d[:])
            idxs = pool.tile([128, K * (SLAB // 16)], mybir.dt.int16, tag="idxs")
            nc.sync.dma_start(idxs[:], i_d[:])

            S16 = SLAB // 16
            k_lo = 0 if gather0 else 1

            def body():
                X = pool.tile([128, wwin], f32, tag="X")
                nc.sync.dma_start(X[:], xwin_d[:])
                Y = pool.tile([128, K * SLAB], f32, tag="Y")
                if not no_gather:
                    if merge_gather:
                        nc.gpsimd.ap_gather(
                            Y[:, k_lo * SLAB:K * SLAB],
                            X[:],
                            idxs[:, k_lo * S16:K * S16],
                            channels=128, num_elems=wwin, d=1,
                            num_idxs=(K - k_lo) * SLAB,
                        )
                    else:
                        for k in range(k_lo, K):
                            nc.gpsimd.ap_gather(
                                Y[:, k * SLAB:(k + 1) * SLAB],
                                X[:],
                                idxs[:, k * S16:(k + 1) * S16],
                                channels=128, num_elems=wwin, d=1, num_idxs=SLAB,
                            )

                for p in range(PAIRS):
                    ps = ppool.tile([128, SLAB], f32, tag=f"ps{p}")
                    for k in range(K):
                        if (k == 0 and not gather0) or no_gather:
                            off = halo if k == 0 else k   # shifted reads in no_gather timing mode
                            rhs = X[64 * p:64 * p + 64, off:off + SLAB]
                        else:
                            rhs = Y[64 * p:64 * p + 64, k * SLAB:(k + 1) * SLAB]
                        lhsT = Wt[64 * p:64 * p + 64, k * 128:(k + 1) * 128]
                        if mm_dtype == "f32r":
                            rhs = rhs.bitcast(mmdt)
                            lhsT = lhsT.bitcast(mmdt)
                        nc.tensor.matmul(ps[:], lhsT=lhsT, rhs=rhs,
                                         start=(k == 0), stop=(k == K - 1))
                    ob = pool.tile([128, SLAB], f32, tag=f"ob{p}")
                    nc.scalar.activation(ob[:], ps[:],
                                         mybir.ActivationFunctionType.Identity,
                                         bias=bias[:])
                    nc.sync.dma_start(o_d[p], ob[:])

            if loop_n:
                with tc.For_i(0, loop_n, 1):
                    body()
            else:
                for _ in range(reps):
                    body()

    nc.compile()
    return nc, wwin


def _prep(idx: np.ndarray, mm_dtype: str):
    key = ("prog", mm_dtype)
    if key in _CACHE:
        return _CACHE[key]
    rel = idx - np.arange(T, dtype=np.int32)[:, None]
    halo = int(max(-rel.min(), rel.max()))
    gather0 = not bool((idx[:, 0] == np.arange(T)).all())
    nc, wwin = _build_program(halo, gather0, mm_dtype)
    _CACHE[key] = (nc, wwin, halo, gather0)
    return _CACHE[key]


# ---------------------------------------------------------------------------
# v2 scheduling rework, same math as v1 (per-k gathers; the f32 tie-breaking
# permutes the k<->offset pairing per token, so the full gather is required):
#   - X window split across two DMA queues (SP + DVE) and issued first; idx on
#     the Pool queue, weights/bias on the Act queue -> gathers start ~2.5us in
#   - matmuls emitted k-major across both pairs so each chases its gather
#   - bias add via DVE tensor_scalar (psum->sbuf, ~2.3x faster than ScalarE
#     activation, and the Act-table load leaves the critical path)
#   - output DMAs on separate queues (SP / Act)


def _build_program_v2(halo: int, gather0: bool, mm_dtype: str):
    import concourse.bacc as bacc
    import concourse.tile as tile
    from concourse import mybir

    wwin = SLAB + 2 * halo
    f32 = mybir.dt.float32
    bf16 = mybir.dt.bfloat16
    # "bf16d": X/Y hold each bf16 value duplicated into a 4-byte unit (so the
    # 4-byte-aligned ap_gather still works); matmuls read stride-2 bf16 views.
    # Otherwise the matmul operand tensors carry the matmul dtype end-to-end
    # (the BIR verifier requires typed producers; DMA cannot round).
    mmdt = {"f32": mybir.dt.float32, "f32r": mybir.dt.float32r,
            "bf16d": f32}[mm_dtype]
    wdt = bf16 if mm_dtype == "bf16d" else mmdt

    nc = bacc.Bacc("TRN2", target_bir_lowering=False, debug=False)
    xwin_d = nc.dram_tensor("xwin", [128, wwin], mmdt, kind="ExternalInput").ap()
    w_d = nc.dram_tensor("wts", [128, K * 128], wdt, kind="ExternalInput").ap()
    b_d = nc.dram_tensor("bias", [128, 1], f32, kind="ExternalInput").ap()
    i_d = nc.dram_tensor("idxs", [128, K * (SLAB // 16)], mybir.dt.int16,
                         kind="ExternalInput").ap()
    o_d = nc.dram_tensor("out", [PAIRS, 128, SLAB], f32,
                         kind="ExternalOutput").ap()

    with tile.TileContext(nc) as tc:
        with (
            tc.tile_pool(name="sb", bufs=1) as pool,
            tc.tile_pool(name="ps", bufs=1, space="PSUM") as ppool,
        ):
            S16 = SLAB // 16
            k_lo = 0 if gather0 else 1
            hw = wwin // 2

            # X first, split across SP + Act queues so the halves transfer
            # concurrently; gathers are gated on X landing.
            X = pool.tile([128, wwin], mmdt, tag="X")
            nc.sync.dma_start(X[:, 0:hw], xwin_d[:, 0:hw])
            nc.scalar.dma_start(X[:, hw:wwin], xwin_d[:, hw:wwin])
            idxs = pool.tile([128, K * S16], mybir.dt.int16, tag="idxs")
            nc.gpsimd.dma_start(idxs[:], i_d[:])
            Wt = pool.tile([128, K * 128], wdt, tag="Wt")
            nc.sync.dma_start(Wt[:], w_d[:])
            bias = pool.tile([128, 1], f32, tag="bias")
            nc.scalar.dma_start(bias[:], b_d[:])

            Y = pool.tile([128, K * SLAB], mmdt, tag="Y")
            for k in range(k_lo, K):
                nc.gpsimd.ap_gather(
                    Y[:, k * SLAB:(k + 1) * SLAB], X[:],
                    idxs[:, k * S16:(k + 1) * S16],
                    channels=128, num_elems=wwin, d=1, num_idxs=SLAB,
                )

            if mm_dtype == "bf16d":
                Xv = X[:].bitcast(bf16).rearrange(
                    "p (w two) -> p w two", two=2)
                Yv = Y[:].bitcast(bf16).rearrange(
                    "p (w two) -> p w two", two=2)
            else:
                Xv = Yv = None

            ps = [ppool.tile([128, SLAB], f32, name=f"ps{p}", tag=f"ps{p}")
                  for p in range(PAIRS)]
            for k in range(K):
                for p in range(PAIRS):
                    if k == 0 and not gather0:
                        if mm_dtype == "bf16d":
                            rhs = Xv[64 * p:64 * p + 64, halo:halo + SLAB, 0]
                        else:
                            rhs = X[64 * p:64 * p + 64, halo:halo + SLAB]
                    else:
                        if mm_dtype == "bf16d":
                            rhs = Yv[64 * p:64 * p + 64,
                                     k * SLAB:(k + 1) * SLAB, 0]
                        else:
                            rhs = Y[64 * p:64 * p + 64, k * SLAB:(k + 1) * SLAB]
                    lhsT = Wt[64 * p:64 * p + 64, k * 128:(k + 1) * 128]
                    nc.tensor.matmul(ps[p][:], lhsT=lhsT, rhs=rhs,
                                     start=(k == 0), stop=(k == K - 1))

            for p in range(PAIRS):
                ob = pool.tile([128, SLAB], f32, tag=f"ob{p}")
                nc.vector.tensor_scalar_add(ob[:], ps[p][:], bias[:])
                if p == 0:
                    nc.sync.dma_start(o_d[p], ob[:])
                else:
                    nc.scalar.dma_start(o_d[p], ob[:])

    nc.compile()
    return nc, wwin


def _to_bf16_bits(a: np.ndarray) -> np.ndarray:
    """f32 -> bf16 bits (uint16), round-to-nearest-even."""
    u = np.ascontiguousarray(a, dtype=np.float32).view(np.uint32)
    return ((u + 0x7FFF + ((u >> 16) & 1)) >> 16).astype(np.uint16)


def _make_in_maps_bf16d(x, conv_w, conv_b, idx, wwin, halo):
    import ml_dtypes
    in_maps = _make_in_maps(x, conv_w, conv_b, idx, wwin, halo)
    wts_bf = in_maps[0]["wts"].astype(ml_dtypes.bfloat16)
    for m in in_maps:
        b = _to_bf16_bits(m["xwin"]).astype(np.uint32)
        m["xwin"] = ((b << 16) | b).view(np.float32)
        m["wts"] = wts_bf
    return in_maps


def _prep_v2(idx: np.ndarray, mm_dtype: str):
    key = ("progv2", mm_dtype)
    if key in _CACHE:
        return _CACHE[key]
    rel = idx - np.arange(T, dtype=np.int32)[:, None]
    halo = int(max(-rel.min(), rel.max()))
    gather0 = not bool((idx[:, 0] == np.arange(T)).all())
    nc, wwin = _build_program_v2(halo, gather0, mm_dtype)
    _CACHE[key] = (nc, wwin, halo)
    return _CACHE[key]


def _make_in_maps(x, conv_w, conv_b, idx, wwin, halo):
    xflat = np.ascontiguousarray(x.reshape(B * C_IN, T), dtype=np.float32)

    # block-diag weights: lhsT_k [64=(bhat,ci), 128=(bhat,co)]
    wT = np.ascontiguousarray(conv_w.transpose(1, 0, 2), dtype=np.float32)  # [ci,co,k]
    wts = np.zeros((64, K, 128), dtype=np.float32)
    for k in range(K):
        wts[0:32, k, 0:64] = wT[:, :, k]
        wts[32:64, k, 64:128] = wT[:, :, k]
    wts = np.concatenate([wts, wts], axis=0).reshape(128, K * 128)

    bias = np.concatenate([conv_b, conv_b]).astype(np.float32)[:, None]

    in_maps = []
    S16 = SLAB // 16
    for g in range(NCORES):
        t0 = g * SLAB
        start = t0 - halo
        xs = np.zeros((128, wwin), dtype=np.float32)
        lo = max(0, start)
        hi = min(T, start + wwin)
        xs[:, lo - start:hi - start] = xflat[:, lo:hi]

        iw = np.zeros((128, K * S16), dtype=np.int16)
        for k in range(K):
            flat = (idx[t0:t0 + SLAB, k] - start).astype(np.int16)
            wrapped = flat.reshape(S16, 16).T          # [16, S16]
            iw[:, k * S16:(k + 1) * S16] = np.tile(wrapped, (8, 1))

        in_maps.append({"xwin": xs, "wts": wts, "bias": bias, "idxs": iw})
    return in_maps


def kernel(x: np.ndarray, conv_w: np.ndarray, conv_b: np.ndarray,
           mm_dtype: str = "bf16d", trace: bool = False,
           mode: str = "v2") -> np.ndarray:
    from concourse.bass_utils import run_bass_kernel_spmd

    x = np.asarray(x, dtype=np.float32)
    conv_w = np.asarray(conv_w, dtype=np.float32)
    conv_b = np.asarray(conv_b, dtype=np.float32)

    idx = _get_idx()
    if mode == "v2":
        nc, wwin, halo = _prep_v2(idx, mm_dtype)
    else:
        nc, wwin, halo, gather0 = _prep(idx, mm_dtype)
    if mode == "v2" and mm_dtype == "bf16d":
        in_maps = _make_in_maps_bf16d(x, conv_w, conv_b, idx, wwin, halo)
    else:
        in_maps = _make_in_maps(x, conv_w, conv_b, idx, wwin, halo)

    res = run_bass_kernel_spmd(nc, in_maps, list(range(NCORES)), trace=trace)
    _CACHE["last_result"] = res

    out = np.empty((B, C_OUT, T), dtype=np.float32)
    for g in range(NCORES):
        o = res.results[g]["out"]          # [PAIRS, 128, SLAB]
        t0 = g * SLAB
        for p in range(PAIRS):
            for bh in range(2):
                out[2 * p + bh, :, t0:t0 + SLAB] = o[p, 64 * bh:64 * bh + 64]
    return out.reshape(B, C_OUT, HH, WW)

